# revision 8
# baseline (speedup 1.0000x reference)
"""Self-contained Trainium2 Bass kernel for a 2-layer GCN (GCNConv -> ReLU -> GCNConv).

Strategy (8 NeuronCores, SPMD):
  - Destination nodes sharded across cores (12544 nodes/core after padding).
  - Host precomputes: symmetric norms, per-core edge slot assignment
    (partition p, chunk column c) per 128-node dst tile, int32 gather indices.
  - Device per core:
      phase 1: h0 = x @ W1 for its node shard (fp16 matmuls, x supplied
               pre-transposed [128, nodes] so features sit on partitions)
      AllGather h0 -> full fp16 gather table in DRAM
      layer 1: per dst tile, indirect-DMA gather of h0[src] rows (128B each),
               one-hot scatter matmul (S[p,j] = (iota==dst_local)*norm built
               on DVE in one dual-op tensor_scalar), PSUM accumulate,
               then +b1, relu -> h shard (fp16)
      AllGather h -> full table
      layer 2: same gather/scatter but transposed (psum[feat, dst]), then
               out = agg @ W2 + b2
  - Host reassembles the 8 output shards.
"""

import numpy as np

import concourse.bacc as bacc
import concourse.bass as bass
import concourse.mybir as mybir
import concourse.tile as tile
from concourse import bass_utils

F16 = mybir.dt.float16
F32 = mybir.dt.float32
I32 = mybir.dt.int32

NCORES = 8
P = 128  # partitions / dst tile size


def preprocess(x, edge_index, W1, b1, W2, b2, n_nodes, group_tiles=14):
    """Host-side graph preprocessing. Returns (in_maps, meta)."""
    n_pad = -(-n_nodes // (NCORES * P)) * (NCORES * P)  # round up to 8*128
    shard = n_pad // NCORES
    tiles = shard // P

    src = np.asarray(edge_index[0], dtype=np.int64)
    dst = np.asarray(edge_index[1], dtype=np.int64)
    loop = np.arange(n_nodes, dtype=np.int64)
    src_f = np.concatenate([src, loop])
    dst_f = np.concatenate([dst, loop])

    deg = np.bincount(dst_f, minlength=n_nodes).astype(np.float32)
    dinv = (1.0 / np.sqrt(np.maximum(deg, 1.0))).astype(np.float32)
    dinv[deg <= 0] = 0.0
    norm = dinv[src] * dinv[dst]

    # group REAL edges (self-loops handled separately via a diagonal matmul
    # on contiguous table rows) by global dst tile
    n_real = len(src)
    gt = dst // P  # global tile id per edge
    order = np.argsort(gt, kind="stable")
    gt_s = gt[order]
    n_gtiles = NCORES * tiles
    counts = np.bincount(gt_s, minlength=n_gtiles)
    starts = np.concatenate([[0], np.cumsum(counts)])
    pos = np.arange(len(order)) - starts[gt_s]

    C = int(-(-counts.max() // P))  # chunks per tile (uniform across cores)
    # make tiles-per-gather-group divide evenly
    while tiles % group_tiles != 0:
        group_tiles -= 1

    cols = tiles * C
    core = gt_s // tiles
    tloc = gt_s % tiles
    cc = pos // P
    pp = pos % P
    col = tloc * C + cc
    flat = core * (P * cols) + pp * cols + col

    idx_all = np.zeros(NCORES * P * cols, np.int32)
    dst_all = np.zeros(NCORES * P * cols, np.float32)
    nrm_all = np.zeros(NCORES * P * cols, np.float32)
    idx_all[flat] = src[order].astype(np.int32)
    dst_all[flat] = (dst[order] % P).astype(np.float32)
    nrm_all[flat] = norm[order]
    idx_all = idx_all.reshape(NCORES, P, cols)
    dst_all = dst_all.reshape(NCORES, P, cols)
    nrm_all = nrm_all.reshape(NCORES, P, cols)

    # self-loop norms dinv^2 laid out [core][p, t] for node k*shard + t*128 + p
    d2 = np.zeros(n_pad, np.float32)
    d2[:n_nodes] = dinv * dinv
    gd2_all = np.ascontiguousarray(
        d2.reshape(NCORES, tiles, P).transpose(0, 2, 1))
    pcol = np.arange(P, dtype=np.float32).reshape(P, 1)

    f_in = x.shape[1]
    xp = np.zeros((n_pad, f_in), np.float16)
    xp[:n_nodes] = x.astype(np.float16)
    xt = np.ascontiguousarray(xp.T)  # [f_in, n_pad]

    w1_16 = np.ascontiguousarray(W1.astype(np.float16))
    hid = W1.shape[1]
    b1bc = np.ascontiguousarray(np.broadcast_to(b1, (P, hid)).astype(np.float32))
    w2_32 = np.ascontiguousarray(W2.astype(np.float32))
    b2sb = np.asarray(b2, np.float32).reshape(1, 1)
    iota = np.ascontiguousarray(
        np.broadcast_to(np.arange(P, dtype=np.float16), (P, P))
    )

    in_maps = []
    for k in range(NCORES):
        in_maps.append(
            {
                "xt": np.ascontiguousarray(xt[:, k * shard : (k + 1) * shard]),
                "w1": w1_16,
                "b1bc": b1bc,
                "w2": w2_32,
                "b2": b2sb,
                "iota": iota,
                "gidx": np.ascontiguousarray(idx_all[k]),
                "gdst": np.ascontiguousarray(dst_all[k]),
                "gnrm": np.ascontiguousarray(nrm_all[k]),
                "gd2": np.ascontiguousarray(gd2_all[k]),
                "pcol": pcol,
            }
        )
    meta = dict(
        n_pad=n_pad, shard=shard, tiles=tiles, C=C, cols=cols,
        group_tiles=group_tiles, f_in=f_in, hid=hid,
    )
    return in_maps, meta


def build_program(meta):
    """Build the SPMD Bass program (identical across cores)."""
    n_pad = meta["n_pad"]
    shard = meta["shard"]
    tiles = meta["tiles"]
    C = meta["C"]
    cols = meta["cols"]
    gtiles = meta["group_tiles"]
    f_in = meta["f_in"]
    hid = meta["hid"]
    ngroups = tiles // gtiles
    gcols = gtiles * C

    nc = bacc.Bacc(None, target_bir_lowering=False, debug=False,
                   num_devices=NCORES)

    xt_d = nc.dram_tensor("xt", [f_in, shard], F16, kind="ExternalInput")
    w1_d = nc.dram_tensor("w1", [f_in, hid], F16, kind="ExternalInput")
    b1_d = nc.dram_tensor("b1bc", [P, hid], F32, kind="ExternalInput")
    w2_d = nc.dram_tensor("w2", [hid, 1], F32, kind="ExternalInput")
    b2_d = nc.dram_tensor("b2", [1, 1], F32, kind="ExternalInput")
    iota_d = nc.dram_tensor("iota", [P, P], F16, kind="ExternalInput")
    gidx_d = nc.dram_tensor("gidx", [P, cols], I32, kind="ExternalInput")
    gdst_d = nc.dram_tensor("gdst", [P, cols], F32, kind="ExternalInput")
    gnrm_d = nc.dram_tensor("gnrm", [P, cols], F32, kind="ExternalInput")
    gd2_d = nc.dram_tensor("gd2", [P, tiles], F32, kind="ExternalInput")
    pcol_d = nc.dram_tensor("pcol", [P, 1], F32, kind="ExternalInput")
    out_d = nc.dram_tensor("out", [1, shard], F32, kind="ExternalOutput")

    rg = [list(range(NCORES))]

    with tile.TileContext(nc) as tc:
        with (
            tc.tile_pool(name="dram", bufs=1, space="DRAM") as dpool,
            tc.tile_pool(name="const", bufs=1) as cpool,
        ):
            h0_shard_d = dpool.tile([shard, hid], F16, tag="h0s", name="h0_shard")
            h0_full_d = dpool.tile([n_pad, hid], F16, tag="h0f",
                                   addr_space="Shared", name="h0_full")
            h_shard_d = dpool.tile([shard, hid], F16, tag="hs", name="h_shard")
            h_full_d = dpool.tile([n_pad, hid], F16, tag="hf",
                                  addr_space="Shared", name="h_full")
            w1_sb = cpool.tile([f_in, hid], F16, tag="w1")
            b1_sb = cpool.tile([P, hid], F32, tag="b1")
            w2_sb = cpool.tile([hid, 1], F32, tag="w2")
            b2_sb = cpool.tile([1, 1], F32, tag="b2")
            iota_sb = cpool.tile([P, P], F16, tag="iota")
            gidx_sb = cpool.tile([P, cols], I32, tag="gidx")
            gdst_sb = cpool.tile([P, cols], F32, tag="gdst")
            gnrm_sb = cpool.tile([P, cols], F32, tag="gnrm")
            gd2_sb = cpool.tile([P, tiles], F32, tag="gd2")
            pcol_sb = cpool.tile([P, 1], F32, tag="pcol")
            outrow_sb = cpool.tile([1, shard], F32, tag="outrow")

            nc.sync.dma_start(w1_sb[:], w1_d[:])
            nc.sync.dma_start(b1_sb[:], b1_d[:])
            nc.sync.dma_start(w2_sb[:], w2_d[:])
            nc.sync.dma_start(b2_sb[:], b2_d[:])
            nc.sync.dma_start(iota_sb[:], iota_d[:])
            nc.sync.dma_start(gidx_sb[:], gidx_d[:])
            nc.sync.dma_start(gdst_sb[:], gdst_d[:])
            nc.sync.dma_start(gnrm_sb[:], gnrm_d[:])
            nc.sync.dma_start(gd2_sb[:], gd2_d[:])
            nc.sync.dma_start(pcol_sb[:], pcol_d[:])

            # ---- phase 1: h0 = x @ W1 on this core's shard ----
            xb = 7 if tiles % 7 == 0 else 1  # node tiles per x DMA
            with (
                tc.tile_pool(name="xt", bufs=3) as xpool,
                tc.tile_pool(name="p1ps", bufs=4, space="PSUM") as p1ps,
                tc.tile_pool(name="h0sb", bufs=4) as h0pool,
            ):
                for tb in range(tiles // xb):
                    xt_t = xpool.tile([f_in, xb * P], F16, tag="xt")
                    nc.sync.dma_start(
                        xt_t[:], xt_d[:, tb * xb * P : (tb + 1) * xb * P]
                    )
                    for i in range(xb):
                        t = tb * xb + i
                        ps = p1ps.tile([P, hid], F32, tag="ps")
                        nc.tensor.matmul(
                            ps[:], xt_t[:, i * P : (i + 1) * P], w1_sb[:],
                            start=True, stop=True,
                        )
                        h0t = h0pool.tile([P, hid], F16, tag="h0t")
                        nc.vector.tensor_copy(h0t[:], ps[:])
                        nc.sync.dma_start(
                            h0_shard_d[t * P : (t + 1) * P, :], h0t[:]
                        )

            nc.gpsimd.collective_compute(
                "AllGather", mybir.AluOpType.bypass, replica_groups=rg,
                ins=[h0_shard_d[:]], outs=[h0_full_d[:]],
            )

            # ---- layer 1: gather h0[src], one-hot scatter matmul ----
            def scatter_layer(table_d, own_d, layer):
                with (
                    tc.tile_pool(name=f"g{layer}", bufs=4) as gpool,
                    tc.tile_pool(name=f"s{layer}", bufs=6) as spool,
                    tc.tile_pool(name=f"ps{layer}", bufs=4, space="PSUM") as pspool,
                    tc.tile_pool(name=f"ep{layer}", bufs=4) as epool,
                    tc.tile_pool(name=f"po{layer}", bufs=2, space="PSUM") as popool,
                ):
                    for t in range(tiles):
                            if layer == 1:
                                ps = pspool.tile([P, hid], F32, tag="ps")
                            else:
                                ps = pspool.tile([hid, P], F32, tag="ps")
                            gt = gpool.tile([P, (C + 1) * hid], F16, tag="G")
                            for c in range(C):
                                col = t * C + c
                                nc.gpsimd.indirect_dma_start(
                                    out=gt[:, c * hid : (c + 1) * hid],
                                    out_offset=None,
                                    in_=table_d[:],
                                    in_offset=bass.IndirectOffsetOnAxis(
                                        ap=gidx_sb[:, col : col + 1],
                                        axis=0,
                                    ),
                                )
                                gslice = gt[:, c * hid : (c + 1) * hid]
                                s = spool.tile([P, P], F16, tag="S")
                                nc.vector.tensor_scalar(
                                    s[:], iota_sb[:],
                                    gdst_sb[:, col : col + 1],
                                    gnrm_sb[:, col : col + 1],
                                    op0=mybir.AluOpType.is_equal,
                                    op1=mybir.AluOpType.mult,
                                )
                                if layer == 1:
                                    nc.tensor.matmul(
                                        ps[:], s[:], gslice,
                                        start=(c == 0), stop=False,
                                    )
                                else:
                                    nc.tensor.matmul(
                                        ps[:], gslice, s[:],
                                        start=(c == 0), stop=False,
                                    )
                            # self-loop contribution: contiguous own rows,
                            # diagonal one-hot with dinv^2
                            hown = gt[:, C * hid : (C + 1) * hid]
                            nc.sync.dma_start(
                                hown, own_d[t * P : (t + 1) * P, :]
                            )
                            sd = spool.tile([P, P], F16, tag="S")
                            nc.vector.tensor_scalar(
                                sd[:], iota_sb[:],
                                pcol_sb[:, 0:1],
                                gd2_sb[:, t : t + 1],
                                op0=mybir.AluOpType.is_equal,
                                op1=mybir.AluOpType.mult,
                            )
                            if layer == 1:
                                nc.tensor.matmul(
                                    ps[:], sd[:], hown,
                                    start=False, stop=True,
                                )
                            else:
                                nc.tensor.matmul(
                                    ps[:], hown, sd[:],
                                    start=False, stop=True,
                                )
                            if layer == 1:
                                tmp = epool.tile([P, hid], F32, tag="tmp")
                                nc.vector.tensor_tensor(
                                    tmp[:], ps[:], b1_sb[:],
                                    op=mybir.AluOpType.add,
                                )
                                ht = epool.tile([P, hid], F16, tag="ht")
                                nc.scalar.activation(
                                    ht[:], tmp[:],
                                    mybir.ActivationFunctionType.Relu,
                                )
                                nc.sync.dma_start(
                                    h_shard_d[t * P : (t + 1) * P, :], ht[:]
                                )
                            else:
                                agg = epool.tile([hid, P], F32, tag="agg")
                                nc.vector.tensor_copy(agg[:], ps[:])
                                po = popool.tile([1, P], F32, tag="po")
                                nc.tensor.matmul(
                                    po[:], w2_sb[:], agg[:],
                                    start=True, stop=True,
                                )
                                nc.vector.tensor_scalar(
                                    outrow_sb[:, t * P : (t + 1) * P],
                                    po[:], b2_sb[0:1, 0:1], None,
                                    op0=mybir.AluOpType.add,
                                )

            scatter_layer(h0_full_d, h0_shard_d, layer=1)

            nc.gpsimd.collective_compute(
                "AllGather", mybir.AluOpType.bypass, replica_groups=rg,
                ins=[h_shard_d[:]], outs=[h_full_d[:]],
            )

            scatter_layer(h_full_d, h_shard_d, layer=2)

            nc.sync.dma_start(out_d[:], outrow_sb[:])

    nc.compile()
    return nc


def kernel(x, edge_index, W1, b1, W2, b2):
    n_nodes = x.shape[0]
    in_maps, meta = preprocess(x, edge_index, W1, b1, W2, b2, n_nodes)
    nc = build_program(meta)
    res = None
    last_err = None
    for _attempt in range(3):
        try:
            res = bass_utils.run_bass_kernel_spmd(
                nc, in_maps, core_ids=list(range(NCORES))
            )
            break
        except Exception as e:  # transient device errors: retry
            last_err = e
            import time as _time
            _time.sleep(5)
    if res is None:
        raise last_err
    outs = [res.results[k]["out"].reshape(-1) for k in range(NCORES)]
    full = np.concatenate(outs)[:n_nodes]
    return full.reshape(n_nodes, 1).astype(np.float32)
